# revision 4
# baseline (speedup 1.0000x reference)
"""Trainium2 Bass kernel for nn_LoRALayer: out = x @ W.T + b + 2.0*(x@A.T)@B.T.

Strategy: 8-way data-parallel over the token dim (N=8192 -> 1024/core).
Per core, a Tile-framework kernel computes the full [1024, 4096] output
shard with fp32r matmuls (full-rate fp32 on the PE at N>=256):

  - x and W are PE-transposed on chip into contraction-major (i-major)
    fp32r tiles (DMA transpose is 2-byte only, so fp32 uses the PE path).
  - The LoRA term and the bias are folded into the same PSUM accumulation
    as the main matmul: per output tile, one extra K=17 matmul with
    lhsT = [2*(x@A.T).T ; ones] and rhs = [B.T ; b].
"""

import os

import numpy as np

try:
    import concourse.bass as bass  # noqa: F401
except ImportError:  # pragma: no cover
    import sys

    sys.path.insert(0, "/opt/trn_rl_repo")
    import concourse.bass as bass  # noqa: F401

import concourse.tile as tile
from concourse import bacc, mybir
from concourse.bass_utils import run_bass_kernel_spmd
from concourse.masks import make_identity

P = 128
N_CORES = 8
N_TOK = 8192
NT = N_TOK // N_CORES  # tokens per core (1024)
KD = 4096  # in_features (contraction)
OD = 4096  # out_features
R = 16
SCALING = 2.0

NS = 256  # out-feature slice per psum group
KT = KD // P  # 32 k-tiles
MT = NT // P  # 8 token tiles per core
NSL = OD // NS  # 16 n slices
ICH = 1024  # natural-layout staging chunk (free dim)

F32 = mybir.dt.float32
F32R = mybir.dt.float32r

_NC_CACHE = None


def _build():
    from contextlib import ExitStack

    nc = bacc.Bacc("TRN2", target_bir_lowering=False, debug=False,
                   num_devices=N_CORES)
    x_d = nc.dram_tensor("x", [NT, KD], F32, kind="ExternalInput").ap()
    w_d = nc.dram_tensor("W", [OD, KD], F32, kind="ExternalInput").ap()
    b_d = nc.dram_tensor("b", [OD], F32, kind="ExternalInput").ap()
    a_d = nc.dram_tensor("lora_A", [R, KD], F32, kind="ExternalInput").ap()
    bb_d = nc.dram_tensor("lora_B", [OD, R], F32, kind="ExternalInput").ap()
    out_d = nc.dram_tensor("out", [NT, OD], F32, kind="ExternalOutput").ap()

    with tile.TileContext(nc) as tc, ExitStack() as ctx:
        const = ctx.enter_context(tc.tile_pool(name="const", bufs=1))
        nat = ctx.enter_context(tc.tile_pool(name="nat", bufs=4))
        xt_pool = ctx.enter_context(tc.tile_pool(name="xt", bufs=KT))
        wt_pool = ctx.enter_context(tc.tile_pool(name="wt", bufs=KT + 4))
        at_pool = ctx.enter_context(tc.tile_pool(name="at", bufs=KT))
        t1_pool = ctx.enter_context(tc.tile_pool(name="t1", bufs=1))
        btb_pool = ctx.enter_context(tc.tile_pool(name="btb", bufs=3))
        osb_pool = ctx.enter_context(tc.tile_pool(name="osb", bufs=3))
        ps_tr = ctx.enter_context(tc.tile_pool(name="ps_tr", bufs=2, space="PSUM"))
        ps_c = ctx.enter_context(tc.tile_pool(name="ps_c", bufs=2, space="PSUM"))
        ps_out = ctx.enter_context(tc.tile_pool(name="ps_out", bufs=4, space="PSUM"))

        ident = const.tile([P, P], F32)
        make_identity(nc, ident[:])

        # b as [128p, 32a]: b[a*128 + p] at (p, a)
        b_all = const.tile([P, OD // P], F32, name="b_all")
        nc.sync.dma_start(b_all[:], b_d.rearrange("(a p) -> p a", p=P))

        # ---- Phase A: lora_A -> AT tiles [128i, 16r] (x SCALING), b rows ----
        at = []
        for ca in range(KD // ICH):
            ach = nat.tile([R, ICH], F32, tag="nat")
            nc.sync.dma_start(ach[:], a_d[:, ca * ICH:(ca + 1) * ICH])
            for j in range(ICH // P):
                pt = ps_tr.tile([P, R], F32, tag="pt")
                nc.tensor.transpose(pt[:], ach[:, j * P:(j + 1) * P],
                                    ident[0:R, 0:R])
                t = at_pool.tile([P, R], F32R, tag="at")
                nc.scalar.mul(t[:], pt[:], SCALING)
                at.append(t)

        # ---- Phase B: x -> xT tiles [128i, 1024t] fp32r (full cache) ----
        xt = [xt_pool.tile([P, NT], F32R, tag="xt", name=f"xt{_k}")
              for _k in range(KT)]
        for mc in range(MT):
            for ic in range(KD // ICH):
                xch = nat.tile([P, ICH], F32, tag="nat")
                nc.sync.dma_start(
                    xch[:], x_d[mc * P:(mc + 1) * P, ic * ICH:(ic + 1) * ICH])
                for j in range(ICH // P):
                    k = ic * (ICH // P) + j
                    pt = ps_tr.tile([P, P], F32, tag="pt")
                    nc.tensor.transpose(pt[:], xch[:, j * P:(j + 1) * P],
                                        ident[:])
                    nc.vector.tensor_copy(xt[k][:, mc * P:(mc + 1) * P], pt[:])

        # ---- Phase C: T1 = [2*(x@A.T).T ; ones] as [17, 1024] fp32r ----
        # Built per token tile: accumulate t1 [128t, 16r] in PSUM, assemble
        # a natural [128t, 32] tile with a ones column, PE-transpose it.
        t1 = t1_pool.tile([32, NT], F32R, tag="t1")
        for m in range(MT):
            pc = ps_c.tile([P, R], F32, tag="pc")
            for k in range(KT):
                nc.tensor.matmul(pc[:], xt[k][:, m * P:(m + 1) * P], at[k][:],
                                 start=(k == 0), stop=(k == KT - 1))
            t1n = nat.tile([P, 32], F32, tag="t1n", bufs=2)
            nc.any.memset(t1n[:], 0.0)
            nc.vector.tensor_copy(t1n[:, 0:R], pc[:])
            nc.any.memset(t1n[:, R:R + 1], 1.0)
            ptr = ps_tr.tile([32, P], F32, tag="pt")
            nc.tensor.transpose(ptr[:], t1n[:], ident[:])
            nc.vector.tensor_copy(t1[:, m * P:(m + 1) * P], ptr[:])

        # ---- Phase D: main loop over out-feature slices ----
        for n in range(NSL):
            o0 = n * NS
            # BTb slice [17, NS]: rows 0..15 = B.T slice, row 16 = b slice.
            # Assemble naturally as [128o, 32] (B chunk | b col | zeros), then
            # PE-transpose so all engine APs start at partition 0.
            btb = btb_pool.tile([32, NS], F32R, tag="btb")
            for g in range(NS // P):
                bn = nat.tile([P, 32], F32, tag="t1n", bufs=2)
                nc.any.memset(bn[:], 0.0)
                nc.sync.dma_start(bn[:, 0:R],
                                  bb_d[o0 + g * P:o0 + (g + 1) * P, :])
                a_idx = (o0 + g * P) // P
                nc.vector.tensor_copy(bn[:, R:R + 1], b_all[:, a_idx:a_idx + 1])
                pt = ps_tr.tile([32, P], F32, tag="pt")
                nc.tensor.transpose(pt[:], bn[:], ident[:])
                nc.vector.tensor_copy(btb[:, g * P:(g + 1) * P], pt[:])

            # W.T slice build: wt_n[k] [128i, NS_o] fp32r
            wt_n = [wt_pool.tile([P, NS], F32R, tag="wt", name=f"wt{n}_{_k}")
                    for _k in range(KT)]
            for g in range(NS // P):
                for ic in range(KD // ICH):
                    wch = nat.tile([P, ICH], F32, tag="nat")
                    nc.sync.dma_start(
                        wch[:],
                        w_d[o0 + g * P:o0 + (g + 1) * P, ic * ICH:(ic + 1) * ICH])
                    for j in range(ICH // P):
                        k = ic * (ICH // P) + j
                        pt = ps_tr.tile([P, P], F32, tag="pt")
                        nc.tensor.transpose(pt[:], wch[:, j * P:(j + 1) * P],
                                            ident[:])
                        nc.vector.tensor_copy(wt_n[k][:, g * P:(g + 1) * P],
                                              pt[:])

            # matmuls: per token tile, accumulate lora+bias then 32 k-tiles
            for m in range(MT):
                po = ps_out.tile([P, NS], F32, tag="po")
                nc.tensor.matmul(po[:], t1[0:R + 1, m * P:(m + 1) * P],
                                 btb[0:R + 1, :], start=True, stop=False)
                for k in range(KT):
                    nc.tensor.matmul(po[:], xt[k][:, m * P:(m + 1) * P],
                                     wt_n[k][:], start=False, stop=(k == KT - 1))
                osb = osb_pool.tile([P, NS], F32, tag="osb")
                nc.scalar.copy(osb[:], po[:])
                nc.sync.dma_start(
                    out_d[m * P:(m + 1) * P, o0:o0 + NS], osb[:])

    nc.compile()
    return nc


def _get_nc():
    global _NC_CACHE
    if _NC_CACHE is None:
        _NC_CACHE = _build()
    return _NC_CACHE


def kernel(x, W, b, lora_A, lora_B):
    nc = _get_nc()
    x = np.ascontiguousarray(x, dtype=np.float32)
    W = np.ascontiguousarray(W, dtype=np.float32)
    b = np.ascontiguousarray(b, dtype=np.float32)
    lora_A = np.ascontiguousarray(lora_A, dtype=np.float32)
    lora_B = np.ascontiguousarray(lora_B, dtype=np.float32)
    in_maps = [
        {
            "x": x[c * NT:(c + 1) * NT],
            "W": W,
            "b": b,
            "lora_A": lora_A,
            "lora_B": lora_B,
        }
        for c in range(N_CORES)
    ]
    res = run_bass_kernel_spmd(nc, in_maps, core_ids=list(range(N_CORES)),
                               trace=bool(int(os.environ.get("LORA_TRACE", "0"))))
    kernel.last_results = res
    return np.concatenate([res.results[c]["out"] for c in range(N_CORES)], axis=0)


if __name__ == "__main__":
    rng = np.random.default_rng(0)
    x = rng.standard_normal((N_TOK, KD), dtype=np.float32)
    W = (rng.standard_normal((OD, KD)) * 0.02).astype(np.float32)
    b = (rng.standard_normal(OD) * 0.02).astype(np.float32)
    A = (rng.standard_normal((R, KD)) * 0.02).astype(np.float32)
    B = (rng.standard_normal((OD, R)) * 0.02).astype(np.float32)
    out = kernel(x=x, W=W, b=b, lora_A=A, lora_B=B)
    ref = x.astype(np.float64) @ W.T.astype(np.float64) + b + SCALING * (
        (x.astype(np.float64) @ A.T.astype(np.float64)) @ B.T.astype(np.float64))
    rel = np.linalg.norm(out - ref) / np.linalg.norm(ref)
    print("rel_l2:", rel)
